# revision 34
# baseline (speedup 1.0000x reference)
"""Masked multi-head self-attention (sparse_attention) on 8 Trainium2 cores.

Strategy
--------
Shard the fused (batch*heads)=16 leading dim of q/k/v across 8 cores, 2 heads
per core.  Per head the kernel computes S^T = K @ Q^T in [j, i] orientation
(128-row j-chunks on partitions, 512-col i-blocks on the free dim), applies
exp on the scalar engine (no max-subtraction needed: |s*scale| <= ~7 so exp
cannot overflow in fp32, and blocked entries are handled structurally, not
additively), then accumulates O^T = V~^T @ P^T on the tensor engine where
V~ = [V | 1] so the softmax denominators fall out of the same matmuls.

The bbox mask has rank-structure: blocked(i,j) <=> (i in A-only and j in
B-only) or vice versa, where A/B are the two subject boxes.  The host sorts
the j (key/value) axis into [A-only | B-only | rest] with 64-aligned zero
padding, so every 64-row half-chunk belongs to one group.  PV matmuls
accumulate into one PSUM accumulator per group; the final combine applies the
per-i 0/1 weights (wA, wB) and sums the three accumulators — the mask costs
no elementwise work on the n*n tiles at all.  Finally each [81, 512] combined
block is PE-transposed back to [128(i), 81], normalized by the sums column,
and DMA'd out in natural i order (the i/query axis is never permuted).
"""

import math
import os

import numpy as np

N_CORES = 8
P = 128  # partitions / j-chunk rows
IB = 512  # i-block width (psum bank, fp32)
DH = 80  # head dim
SUM_ROW = 96  # 32-aligned partition for the sums row (DVE slice rule)
DV = SUM_ROW + 1  # V padded to 96, plus the ones column

_PROGRAM_CACHE = {}
LAST_RESULTS = None  # BassKernelResults of the most recent run (for test.py)


# ----------------------------------------------------------------------------
# host-side mask analysis (mirrors reference._subject_masks / _self_mask)
# ----------------------------------------------------------------------------

def _subject_masks_np(bboxes: np.ndarray, resolution: int) -> np.ndarray:
    b = bboxes[0].astype(np.float32)  # [s, 4]
    x0 = np.round(b[:, 0] * resolution)
    y0 = np.round(b[:, 1] * resolution)
    x1 = np.round(b[:, 2] * resolution)
    y1 = np.round(b[:, 3] * resolution)
    coords = np.arange(resolution, dtype=np.float32)
    xm = (coords[None, :] >= x0[:, None]) & (coords[None, :] < x1[:, None])
    ym = (coords[None, :] >= y0[:, None]) & (coords[None, :] < y1[:, None])
    return (ym[:, :, None] & xm[:, None, :]).reshape(b.shape[0], -1)  # [s, n]


def _group_layout(bboxes: np.ndarray, n: int):
    """Sort the j axis into [A-only | B-only | rest], 64-aligned groups.

    Returns (perm, seg_sizes, group_starts, n_pad, wA, wB) where perm is the
    source index for each padded slot (-1 for zero padding), wA/wB are the
    per-original-i {0,1} combine weights for the A/B accumulators.
    """
    res = int(math.isqrt(n))
    assert res * res == n
    subj = _subject_masks_np(bboxes, res)
    assert subj.shape[0] == 2, "kernel specialized for 2 subject boxes"
    m0, m1 = subj[0], subj[1]
    e0 = m0 & ~m1  # A-only
    e1 = m1 & ~m0  # B-only
    rest = ~(e0 | e1)

    idx = np.arange(n)
    groups = [idx[e0], idx[e1], idx[rest]]

    def ceil64(x):
        return ((x + 63) // 64) * 64

    padded = [ceil64(len(g)) for g in groups]
    n_pad = sum(padded)
    if n_pad % P:
        padded[2] += 64
        n_pad += 64
    perm = np.full(n_pad, -1, dtype=np.int64)
    starts = []
    pos = 0
    for g, plen in zip(groups, padded):
        starts.append(pos)
        perm[pos : pos + len(g)] = g
        pos += plen

    wA = (~e1).astype(np.float32)  # zero the A accumulator for i in B-only
    wB = (~e0).astype(np.float32)  # zero the B accumulator for i in A-only
    return perm, padded, starts, n_pad, wA, wB


def _chunk_segments(padded, starts, n_pad):
    """Per 128-chunk: list of (row_lo, row_hi, group_id) 64-aligned segments."""
    half_group = np.empty(n_pad // 64, dtype=np.int64)
    for gid, (st, plen) in enumerate(zip(starts, padded)):
        half_group[st // 64 : (st + plen) // 64] = gid
    segments = []
    for c in range(n_pad // P):
        g0 = int(half_group[2 * c])
        g1 = int(half_group[2 * c + 1])
        if g0 == g1:
            segments.append([(0, P, g0)])
        else:
            segments.append([(0, 64, g0), (64, P, g1)])
    return segments


# ----------------------------------------------------------------------------
# device program
# ----------------------------------------------------------------------------

def _build_program(n, n_pad, heads_per_core, segments, present_groups, scale):
    import concourse.mybir as mybir
    import concourse.tile as tile
    from concourse import bacc

    f32 = mybir.dt.float32
    f32r = mybir.dt.float32r
    nch = n_pad // P
    n_ib = n // IB
    Exp = mybir.ActivationFunctionType.Exp
    MUL = mybir.AluOpType.mult
    ADD = mybir.AluOpType.add

    nc = bacc.Bacc("TRN2", target_bir_lowering=False, debug=False,
                   num_devices=N_CORES)
    qT_d = nc.dram_tensor("qT", [heads_per_core, DH, n], f32r, kind="ExternalInput")
    kT_d = nc.dram_tensor("kT", [heads_per_core, DH, n_pad], f32r,
                          kind="ExternalInput")
    vt_d = nc.dram_tensor("vt", [heads_per_core, n_pad, DV], f32r,
                          kind="ExternalInput")
    wab_d = nc.dram_tensor("wab", [DV, n], f32, kind="ExternalInput")
    wbb_d = nc.dram_tensor("wbb", [DV, n], f32, kind="ExternalInput")
    ones_d = nc.dram_tensor("ones", [1, DH], f32, kind="ExternalInput")
    oT_d = nc.dram_tensor("oT", [heads_per_core, DH, n], f32,
                          kind="ExternalOutput")

    # Chunk processing order: a pure-neutral singleton first (short
    # cross-i-block dependency chain), then every chunk touching the A/B
    # accumulators (so those accumulators finish early and the combine
    # overlaps the long neutral tail), then the remaining neutral chunks.
    ab_chunks = [c for c, segs in enumerate(segments)
                 if any(g != 2 for (_, _, g) in segs)]
    order = []
    rest = list(range(nch))
    if nch % 2:
        order.append((rest.pop(0),))
    order += [tuple(rest[i : i + 2]) for i in range(0, len(rest), 2)]
    last_ab_pair = None  # partial combine disabled (bisect)

    # first/last (chunk, row) PV matmul per group in traversal order
    first_seg = {}
    last_seg = {}
    for pr in order:
        for c in pr:
            for (r0, _, g) in segments[c]:
                first_seg.setdefault(g, (c, r0))
                last_seg[g] = (c, r0)

    with tile.TileContext(nc) as tc:
        with (
            tc.tile_pool(name="const", bufs=1) as const_pool,
            tc.tile_pool(name="head", bufs=2) as head_pool,
            tc.tile_pool(name="p", bufs=3) as p_pool,
            tc.tile_pool(name="comb", bufs=2) as comb_pool,
            tc.tile_pool(name="out", bufs=4) as out_pool,
            tc.tile_pool(name="s_ps", bufs=2, space="PSUM") as s_pool,
            tc.tile_pool(name="acc_ps", bufs=1, space="PSUM") as acc_pool,
            tc.tile_pool(name="bc_ps", bufs=1, space="PSUM") as bc_pool,
        ):
            wab_t = const_pool.tile([DV, n], f32)
            wbb_t = const_pool.tile([DV, n], f32)
            ones_t = const_pool.tile([1, DH], f32)
            nc.sync.dma_start(ones_t[:], ones_d[:])

            # pre-warm the exp table set while the first DMAs run
            warm = const_pool.tile([P, 1], f32)
            nc.vector.memset(warm[:], 0.0)
            nc.scalar.activation(warm[:], warm[:], Exp)

            def load_head(h):
                kT_t = head_pool.tile([DH, nch, P], f32r, tag="kT",
                                      name=f"kT_{h}")
                qT_t = head_pool.tile([DH, n], f32r, tag="qT", name=f"qT_{h}")
                vt_t = head_pool.tile([P, nch, DV], f32r, tag="vt",
                                      name=f"vt_{h}")
                kT_src = kT_d[h].rearrange("d (c j) -> d c j", j=P)
                vt_src = vt_d[h].rearrange("(c p) d -> p c d", p=P)
                # traversal-ordered slices; first slice covers the first pairs
                lead = sorted(set(order[0] + order[1]))
                hi = max(lead) + 1
                cuts = [0, hi]
                for c in (hi + 4, hi + 10, hi + 18, nch):
                    if c > cuts[-1] and c <= nch:
                        cuts.append(min(c, nch))
                if cuts[-1] != nch:
                    cuts.append(nch)
                nc.sync.dma_start(kT_t[:, 0:hi, :], kT_src[:, 0:hi, :])
                nc.sync.dma_start(qT_t[:, 0:IB], qT_d[h][:, 0:IB])
                nc.sync.dma_start(vt_t[:, 0:hi, :], vt_src[:, 0:hi, :])
                ib_next = 1
                for c0, c1 in zip(cuts[1:], cuts[2:]):
                    nc.sync.dma_start(kT_t[:, c0:c1, :], kT_src[:, c0:c1, :])
                    nc.sync.dma_start(vt_t[:, c0:c1, :], vt_src[:, c0:c1, :])
                    if ib_next < n_ib:
                        nc.sync.dma_start(
                            qT_t[:, ib_next * IB : (ib_next + 1) * IB],
                            qT_d[h][:, ib_next * IB : (ib_next + 1) * IB])
                        ib_next += 1
                for ib2 in range(ib_next, n_ib):
                    nc.sync.dma_start(qT_t[:, ib2 * IB : (ib2 + 1) * IB],
                                      qT_d[h][:, ib2 * IB : (ib2 + 1) * IB])
                return kT_t, qT_t, vt_t

            head_tiles = {0: load_head(0)}

            pending_epilogue = None
            pending_epilogue_b = None
            pending_pv = None
            consts_loaded = [False]

            for h in range(heads_per_core):
                kT_t, qT_t, vt_t = head_tiles[h]

                for ib in range(n_ib):
                    if ib == 4 and h + 1 < heads_per_core:
                        head_tiles[h + 1] = load_head(h + 1)
                    accs = {
                        g: acc_pool.tile([DV, IB], f32, tag=f"acc{g}",
                                         name=f"acc{g}_{h}_{ib}")
                        for g in present_groups
                    }
                    cell = {}
                    q_sl = qT_t[:, ib * IB : (ib + 1) * IB]

                    def make_partial(accs=accs, h=h, ib=ib, cell=cell):
                        def partial():
                            # A/B accumulators are final: fold them with the
                            # per-i weights now, overlapping the neutral tail
                            i_sl = slice(ib * IB, (ib + 1) * IB)
                            t12 = None
                            if 0 in accs:
                                t1 = comb_pool.tile([DV, IB], f32, tag="t1",
                                                    name=f"t1_{h}_{ib}")
                                nc.vector.tensor_tensor(
                                    t1[:], accs[0][:], wab_t[:, i_sl], op=MUL)
                                t12 = t1
                            if 1 in accs:
                                t2 = comb_pool.tile([DV, IB], f32, tag="t2",
                                                    name=f"t2_{h}_{ib}")
                                nc.vector.tensor_tensor(
                                    t2[:], accs[1][:], wbb_t[:, i_sl], op=MUL)
                                if t12 is None:
                                    t12 = t2
                                else:
                                    nc.vector.tensor_tensor(t12[:], t12[:],
                                                            t2[:], op=ADD)
                            cell["t12"] = t12
                        return partial

                    pending_partial = (make_partial()
                                       if last_ab_pair is not None else None)
                    if pending_partial is None:
                        cell["t12"] = None

                    for t, pr in enumerate(order):
                        if t == 2 and not consts_loaded[0]:
                            # combine weights: needed first by the partial
                            # combine a few pairs from now; off the critical
                            # first-chunk DMAs
                            nc.sync.dma_start(wab_t[:], wab_d[:])
                            nc.sync.dma_start(wbb_t[:], wbb_d[:])
                            consts_loaded[0] = True
                        s_t = s_pool.tile([P, IB * len(pr)], f32, tag="s")
                        for pi, c in enumerate(pr):
                            nc.tensor.matmul(
                                s_t[:, pi * IB : (pi + 1) * IB],
                                lhsT=kT_t[:, c, :],
                                rhs=q_sl,
                                start=True,
                                stop=True,
                            )
                        p_t = p_pool.tile([P, IB * len(pr)], f32r, tag="p")
                        nc.scalar.activation(p_t[:], s_t[:], Exp, scale=scale)
                        if pending_pv is not None:
                            pending_pv()
                            pending_pv = None
                            if (pending_partial is not None
                                    and t == last_ab_pair + 1):
                                pending_partial()
                                pending_partial = None
                        if t == 2 and pending_epilogue is not None:
                            pending_epilogue()
                            pending_epilogue = None
                        elif t == 4 and pending_epilogue_b is not None:
                            pending_epilogue_b()
                            pending_epilogue_b = None

                        def make_pv(pr=pr, p_t=p_t, accs=accs, vt_t=vt_t):
                            def pv():
                                for pi, c in enumerate(pr):
                                    for (r0, r1, g) in segments[c]:
                                        nc.tensor.matmul(
                                            accs[g][:],
                                            lhsT=vt_t[r0:r1, c, :],
                                            rhs=p_t[r0:r1,
                                                    pi * IB : (pi + 1) * IB],
                                            start=((c, r0) == first_seg[g]),
                                            stop=((c, r0) == last_seg[g]),
                                        )
                            return pv

                        pending_pv = make_pv()

                    if pending_partial is not None:
                        # A/B tail reached the end of the block; flush the lag
                        pending_pv()
                        pending_pv = None
                        pending_partial()
                        pending_partial = None

                    def make_epilogue_a(accs=accs, h=h, ib=ib, cell=cell):
                        def epilogue_a():
                            comb = comb_pool.tile([DV, IB], f32, tag="comb",
                                                  name=f"comb_{h}_{ib}")
                            t12 = cell["t12"]
                            i_sl = slice(ib * IB, (ib + 1) * IB)
                            if t12 is None and (0 in accs or 1 in accs):
                                t1 = comb_pool.tile([DV, IB], f32, tag="t1",
                                                    name=f"t1f_{h}_{ib}")
                                parts = []
                                if 0 in accs:
                                    nc.vector.tensor_tensor(
                                        t1[:], accs[0][:], wab_t[:, i_sl],
                                        op=MUL)
                                    parts.append(t1)
                                if 1 in accs:
                                    t2 = comb_pool.tile(
                                        [DV, IB], f32, tag="t2",
                                        name=f"t2f_{h}_{ib}")
                                    nc.vector.tensor_tensor(
                                        t2[:], accs[1][:], wbb_t[:, i_sl],
                                        op=MUL)
                                    if parts:
                                        nc.vector.tensor_tensor(
                                            t1[:], t1[:], t2[:], op=ADD)
                                    else:
                                        parts.append(t2)
                                        t1 = t2
                                nc.vector.tensor_tensor(comb[:], t1[:],
                                                        accs[2][:], op=ADD)
                            elif t12 is not None:
                                nc.vector.tensor_tensor(comb[:], t12[:],
                                                        accs[2][:], op=ADD)
                            else:
                                nc.vector.tensor_copy(comb[:], accs[2][:])
                            rrow = out_pool.tile([1, IB], f32, tag="rrow",
                                                 name=f"rrow_{h}_{ib}")
                            nc.vector.reciprocal(
                                rrow[:], comb[SUM_ROW : SUM_ROW + 1, :])
                            cell["comb"] = comb
                            cell["rrow"] = rrow
                        return epilogue_a

                    def make_epilogue_b(h=h, ib=ib, cell=cell):
                        def epilogue_b():
                            comb, rrow = cell["comb"], cell["rrow"]
                            bc = bc_pool.tile([DH, IB], f32, tag="bc",
                                              name=f"bc_{h}_{ib}")
                            nc.tensor.matmul(bc[:], lhsT=ones_t[:], rhs=rrow[:],
                                             start=True, stop=True)
                            o_sb = out_pool.tile([DH, IB], f32, tag="osb",
                                                 name=f"osb_{h}_{ib}")
                            nc.vector.tensor_tensor(o_sb[:], comb[:DH, :],
                                                    bc[:], op=MUL)
                            nc.sync.dma_start(
                                oT_d[h, :, ib * IB : (ib + 1) * IB], o_sb[:])
                        return epilogue_b

                    # flush leftovers (only reachable when pairs-per-block
                    # is small, e.g. tiny-n debug configs)
                    if pending_epilogue is not None:
                        pending_epilogue()
                    if pending_epilogue_b is not None:
                        pending_epilogue_b()
                    pending_epilogue = make_epilogue_a()
                    pending_epilogue_b = make_epilogue_b()

            if pending_pv is not None:
                pending_pv()
            if pending_epilogue is not None:
                pending_epilogue()
            if pending_epilogue_b is not None:
                pending_epilogue_b()

    nc.compile()
    return nc


# ----------------------------------------------------------------------------
# entry point
# ----------------------------------------------------------------------------

def kernel(hidden_states, q, k, v, bboxes, is_cross, ith, num_heads):
    global LAST_RESULTS
    if is_cross:
        return np.asarray(hidden_states)

    from concourse.bass_utils import run_bass_kernel_spmd

    q = np.ascontiguousarray(np.asarray(q, dtype=np.float32))
    k = np.ascontiguousarray(np.asarray(k, dtype=np.float32))
    v = np.ascontiguousarray(np.asarray(v, dtype=np.float32))
    bboxes = np.asarray(bboxes, dtype=np.float32)
    num_heads = int(num_heads)

    bh, n, dh = q.shape
    assert dh == DH and bh % N_CORES == 0 and n % IB == 0
    heads_per_core = bh // N_CORES
    batch = bh // num_heads
    scale = float(1.0 / np.sqrt(np.float32(dh)))

    perm, padded, starts, n_pad, wA, wB = _group_layout(bboxes, n)
    segments = _chunk_segments(padded, starts, n_pad)
    present_groups = sorted({g for segs in segments for (_, _, g) in segs})

    key = (n, n_pad, heads_per_core, tuple(tuple(s) for s in segments))
    if key not in _PROGRAM_CACHE:
        _PROGRAM_CACHE[key] = _build_program(
            n, n_pad, heads_per_core, segments, present_groups, scale
        )
    nc = _PROGRAM_CACHE[key]

    # host-side input prep
    sel = perm >= 0
    kp = np.zeros((bh, n_pad, dh), np.float32)
    kp[:, sel, :] = k[:, perm[sel], :]
    vt = np.zeros((bh, n_pad, DV), np.float32)
    vt[:, sel, :dh] = v[:, perm[sel], :]
    vt[:, sel, SUM_ROW] = 1.0
    kT = np.ascontiguousarray(kp.transpose(0, 2, 1))  # [bh, dh, n_pad]
    qT = np.ascontiguousarray(q.transpose(0, 2, 1))  # [bh, dh, n]
    wab = np.ascontiguousarray(np.broadcast_to(wA[None, :], (DV, n)))
    wbb = np.ascontiguousarray(np.broadcast_to(wB[None, :], (DV, n)))

    in_maps = []
    for c in range(N_CORES):
        sl = slice(c * heads_per_core, (c + 1) * heads_per_core)
        in_maps.append({
            "qT": qT[sl], "kT": kT[sl], "vt": vt[sl],
            "wab": wab, "wbb": wbb, "ones": np.ones((1, DH), np.float32),
        })

    trace = bool(int(os.environ.get("BASS_ATTN_TRACE", "0")))
    kwargs = {}
    if trace:
        kwargs = dict(trace=True, trace_cores=list(range(N_CORES)))
    res = run_bass_kernel_spmd(nc, in_maps, core_ids=list(range(N_CORES)), **kwargs)
    LAST_RESULTS = res

    out = np.empty((batch, n, num_heads * dh), np.float32)
    for bh_idx in range(bh):
        c, hh = divmod(bh_idx, heads_per_core)
        b, hd = divmod(bh_idx, num_heads)
        out[b, :, hd * dh : (hd + 1) * dh] = res.results[c]["oT"][hh].T
    return out


# revision 35
# speedup vs baseline: 1.1624x; 1.1624x over previous
"""Masked multi-head self-attention (sparse_attention) on 8 Trainium2 cores.

Strategy
--------
Shard the fused (batch*heads)=16 leading dim of q/k/v across 8 cores, 2 heads
per core.  Per head the kernel computes S^T = K @ Q^T in [j, i] orientation
(128-row j-chunks on partitions, 512-col i-blocks on the free dim), applies
exp on the scalar engine (no max-subtraction needed: |s*scale| <= ~7 so exp
cannot overflow in fp32, and blocked entries are handled structurally, not
additively), then accumulates O^T = V~^T @ P^T on the tensor engine where
V~ = [V | 1] so the softmax denominators fall out of the same matmuls.

The bbox mask has rank-structure: blocked(i,j) <=> (i in A-only and j in
B-only) or vice versa, where A/B are the two subject boxes.  The host sorts
the j (key/value) axis into [A-only | B-only | rest] with 64-aligned zero
padding, so every 64-row half-chunk belongs to one group.  PV matmuls
accumulate into one PSUM accumulator per group; the final combine applies the
per-i 0/1 weights (wA, wB) and sums the three accumulators — the mask costs
no elementwise work on the n*n tiles at all.  Finally each [81, 512] combined
block is PE-transposed back to [128(i), 81], normalized by the sums column,
and DMA'd out in natural i order (the i/query axis is never permuted).
"""

import math
import os

import numpy as np

N_CORES = 8
P = 128  # partitions / j-chunk rows
IB = 512  # i-block width (psum bank, fp32)
DH = 80  # head dim
SUM_ROW = 96  # 32-aligned partition for the sums row (DVE slice rule)
DV = SUM_ROW + 1  # V padded to 96, plus the ones column

_PROGRAM_CACHE = {}
LAST_RESULTS = None  # BassKernelResults of the most recent run (for test.py)


# ----------------------------------------------------------------------------
# host-side mask analysis (mirrors reference._subject_masks / _self_mask)
# ----------------------------------------------------------------------------

def _subject_masks_np(bboxes: np.ndarray, resolution: int) -> np.ndarray:
    b = bboxes[0].astype(np.float32)  # [s, 4]
    x0 = np.round(b[:, 0] * resolution)
    y0 = np.round(b[:, 1] * resolution)
    x1 = np.round(b[:, 2] * resolution)
    y1 = np.round(b[:, 3] * resolution)
    coords = np.arange(resolution, dtype=np.float32)
    xm = (coords[None, :] >= x0[:, None]) & (coords[None, :] < x1[:, None])
    ym = (coords[None, :] >= y0[:, None]) & (coords[None, :] < y1[:, None])
    return (ym[:, :, None] & xm[:, None, :]).reshape(b.shape[0], -1)  # [s, n]


def _group_layout(bboxes: np.ndarray, n: int):
    """Sort the j axis into [A-only | B-only | rest], 64-aligned groups.

    Returns (perm, seg_sizes, group_starts, n_pad, wA, wB) where perm is the
    source index for each padded slot (-1 for zero padding), wA/wB are the
    per-original-i {0,1} combine weights for the A/B accumulators.
    """
    res = int(math.isqrt(n))
    assert res * res == n
    subj = _subject_masks_np(bboxes, res)
    assert subj.shape[0] == 2, "kernel specialized for 2 subject boxes"
    m0, m1 = subj[0], subj[1]
    e0 = m0 & ~m1  # A-only
    e1 = m1 & ~m0  # B-only
    rest = ~(e0 | e1)

    idx = np.arange(n)
    groups = [idx[e0], idx[e1], idx[rest]]

    def ceil64(x):
        return ((x + 63) // 64) * 64

    padded = [ceil64(len(g)) for g in groups]
    n_pad = sum(padded)
    if n_pad % P:
        padded[2] += 64
        n_pad += 64
    perm = np.full(n_pad, -1, dtype=np.int64)
    starts = []
    pos = 0
    for g, plen in zip(groups, padded):
        starts.append(pos)
        perm[pos : pos + len(g)] = g
        pos += plen

    wA = (~e1).astype(np.float32)  # zero the A accumulator for i in B-only
    wB = (~e0).astype(np.float32)  # zero the B accumulator for i in A-only
    return perm, padded, starts, n_pad, wA, wB


def _chunk_segments(padded, starts, n_pad):
    """Per 128-chunk: list of (row_lo, row_hi, group_id) 64-aligned segments."""
    half_group = np.empty(n_pad // 64, dtype=np.int64)
    for gid, (st, plen) in enumerate(zip(starts, padded)):
        half_group[st // 64 : (st + plen) // 64] = gid
    segments = []
    for c in range(n_pad // P):
        g0 = int(half_group[2 * c])
        g1 = int(half_group[2 * c + 1])
        if g0 == g1:
            segments.append([(0, P, g0)])
        else:
            segments.append([(0, 64, g0), (64, P, g1)])
    return segments


# ----------------------------------------------------------------------------
# device program
# ----------------------------------------------------------------------------

def _build_program(n, n_pad, heads_per_core, segments, present_groups, scale):
    import concourse.mybir as mybir
    import concourse.tile as tile
    from concourse import bacc

    f32 = mybir.dt.float32
    f32r = mybir.dt.float32r
    nch = n_pad // P
    n_ib = n // IB
    Exp = mybir.ActivationFunctionType.Exp
    MUL = mybir.AluOpType.mult
    ADD = mybir.AluOpType.add

    nc = bacc.Bacc("TRN2", target_bir_lowering=False, debug=False,
                   num_devices=N_CORES)
    qT_d = nc.dram_tensor("qT", [heads_per_core, DH, n], f32r, kind="ExternalInput")
    kT_d = nc.dram_tensor("kT", [heads_per_core, DH, n_pad], f32r,
                          kind="ExternalInput")
    vt_d = nc.dram_tensor("vt", [heads_per_core, n_pad, DV], f32r,
                          kind="ExternalInput")
    wab_d = nc.dram_tensor("wab", [DV, n], f32, kind="ExternalInput")
    wbb_d = nc.dram_tensor("wbb", [DV, n], f32, kind="ExternalInput")
    id_d = nc.dram_tensor("ident", [P, P], f32, kind="ExternalInput")
    o_d = nc.dram_tensor("o", [heads_per_core, n, DH], f32,
                         kind="ExternalOutput")

    # Chunk processing order: a pure-neutral singleton first (short
    # cross-i-block dependency chain), then every chunk touching the A/B
    # accumulators (so those accumulators finish early and the combine
    # overlaps the long neutral tail), then the remaining neutral chunks.
    ab_chunks = [c for c, segs in enumerate(segments)
                 if any(g != 2 for (_, _, g) in segs)]
    order = []
    rest = list(range(nch))
    if nch % 2:
        order.append((rest.pop(0),))
    order += [tuple(rest[i : i + 2]) for i in range(0, len(rest), 2)]
    last_ab_pair = None  # partial combine disabled (bisect)

    # first/last (chunk, row) PV matmul per group in traversal order
    first_seg = {}
    last_seg = {}
    for pr in order:
        for c in pr:
            for (r0, _, g) in segments[c]:
                first_seg.setdefault(g, (c, r0))
                last_seg[g] = (c, r0)

    with tile.TileContext(nc) as tc:
        with (
            tc.tile_pool(name="const", bufs=1) as const_pool,
            tc.tile_pool(name="head", bufs=2) as head_pool,
            tc.tile_pool(name="p", bufs=3) as p_pool,
            tc.tile_pool(name="comb", bufs=2) as comb_pool,
            tc.tile_pool(name="out", bufs=4) as out_pool,
            tc.tile_pool(name="s_ps", bufs=2, space="PSUM") as s_pool,
            tc.tile_pool(name="acc_ps", bufs=1, space="PSUM") as acc_pool,
            tc.tile_pool(name="tr_ps", bufs=1, space="PSUM") as tr_pool,
        ):
            wab_t = const_pool.tile([DV, n], f32)
            wbb_t = const_pool.tile([DV, n], f32)
            ident = const_pool.tile([P, P], f32)
            nc.sync.dma_start(ident[:], id_d[:])

            # pre-warm the exp table set while the first DMAs run
            warm = const_pool.tile([P, 1], f32)
            nc.vector.memset(warm[:], 0.0)
            nc.scalar.activation(warm[:], warm[:], Exp)

            def load_head(h):
                kT_t = head_pool.tile([DH, nch, P], f32r, tag="kT",
                                      name=f"kT_{h}")
                qT_t = head_pool.tile([DH, n], f32r, tag="qT", name=f"qT_{h}")
                vt_t = head_pool.tile([P, nch, DV], f32r, tag="vt",
                                      name=f"vt_{h}")
                kT_src = kT_d[h].rearrange("d (c j) -> d c j", j=P)
                vt_src = vt_d[h].rearrange("(c p) d -> p c d", p=P)
                # traversal-ordered slices; first slice covers the first pairs
                lead = sorted(set(order[0] + order[1]))
                hi = max(lead) + 1
                cuts = [0, hi]
                for c in (hi + 4, hi + 10, hi + 18, nch):
                    if c > cuts[-1] and c <= nch:
                        cuts.append(min(c, nch))
                if cuts[-1] != nch:
                    cuts.append(nch)
                nc.sync.dma_start(kT_t[:, 0:hi, :], kT_src[:, 0:hi, :])
                nc.sync.dma_start(qT_t[:, 0:IB], qT_d[h][:, 0:IB])
                nc.sync.dma_start(vt_t[:, 0:hi, :], vt_src[:, 0:hi, :])
                ib_next = 1
                for c0, c1 in zip(cuts[1:], cuts[2:]):
                    nc.sync.dma_start(kT_t[:, c0:c1, :], kT_src[:, c0:c1, :])
                    nc.sync.dma_start(vt_t[:, c0:c1, :], vt_src[:, c0:c1, :])
                    if ib_next < n_ib:
                        nc.sync.dma_start(
                            qT_t[:, ib_next * IB : (ib_next + 1) * IB],
                            qT_d[h][:, ib_next * IB : (ib_next + 1) * IB])
                        ib_next += 1
                for ib2 in range(ib_next, n_ib):
                    nc.sync.dma_start(qT_t[:, ib2 * IB : (ib2 + 1) * IB],
                                      qT_d[h][:, ib2 * IB : (ib2 + 1) * IB])
                return kT_t, qT_t, vt_t

            head_tiles = {0: load_head(0)}

            pending_epilogue = None
            pending_epilogue_b = None
            pending_pv = None
            consts_loaded = [False]

            for h in range(heads_per_core):
                kT_t, qT_t, vt_t = head_tiles[h]

                for ib in range(n_ib):
                    if ib == 4 and h + 1 < heads_per_core:
                        head_tiles[h + 1] = load_head(h + 1)
                    accs = {
                        g: acc_pool.tile([DV, IB], f32, tag=f"acc{g}",
                                         name=f"acc{g}_{h}_{ib}")
                        for g in present_groups
                    }
                    cell = {}
                    q_sl = qT_t[:, ib * IB : (ib + 1) * IB]

                    def make_partial(accs=accs, h=h, ib=ib, cell=cell):
                        def partial():
                            # A/B accumulators are final: fold them with the
                            # per-i weights now, overlapping the neutral tail
                            i_sl = slice(ib * IB, (ib + 1) * IB)
                            t12 = None
                            if 0 in accs:
                                t1 = comb_pool.tile([DV, IB], f32, tag="t1",
                                                    name=f"t1_{h}_{ib}")
                                nc.vector.tensor_tensor(
                                    t1[:], accs[0][:], wab_t[:, i_sl], op=MUL)
                                t12 = t1
                            if 1 in accs:
                                t2 = comb_pool.tile([DV, IB], f32, tag="t2",
                                                    name=f"t2_{h}_{ib}")
                                nc.vector.tensor_tensor(
                                    t2[:], accs[1][:], wbb_t[:, i_sl], op=MUL)
                                if t12 is None:
                                    t12 = t2
                                else:
                                    nc.vector.tensor_tensor(t12[:], t12[:],
                                                            t2[:], op=ADD)
                            cell["t12"] = t12
                        return partial

                    pending_partial = (make_partial()
                                       if last_ab_pair is not None else None)
                    if pending_partial is None:
                        cell["t12"] = None

                    for t, pr in enumerate(order):
                        if t == 2 and not consts_loaded[0]:
                            # combine weights: needed first by the partial
                            # combine a few pairs from now; off the critical
                            # first-chunk DMAs
                            nc.sync.dma_start(wab_t[:], wab_d[:])
                            nc.sync.dma_start(wbb_t[:], wbb_d[:])
                            consts_loaded[0] = True
                        s_t = s_pool.tile([P, IB * len(pr)], f32, tag="s")
                        for pi, c in enumerate(pr):
                            nc.tensor.matmul(
                                s_t[:, pi * IB : (pi + 1) * IB],
                                lhsT=kT_t[:, c, :],
                                rhs=q_sl,
                                start=True,
                                stop=True,
                            )
                        p_t = p_pool.tile([P, IB * len(pr)], f32r, tag="p")
                        nc.scalar.activation(p_t[:], s_t[:], Exp, scale=scale)
                        if pending_pv is not None:
                            pending_pv()
                            pending_pv = None
                            if (pending_partial is not None
                                    and t == last_ab_pair + 1):
                                pending_partial()
                                pending_partial = None
                        if t == 2 and pending_epilogue is not None:
                            pending_epilogue()
                            pending_epilogue = None
                        elif t == 4 and pending_epilogue_b is not None:
                            pending_epilogue_b()
                            pending_epilogue_b = None

                        def make_pv(pr=pr, p_t=p_t, accs=accs, vt_t=vt_t):
                            def pv():
                                for pi, c in enumerate(pr):
                                    for (r0, r1, g) in segments[c]:
                                        nc.tensor.matmul(
                                            accs[g][:],
                                            lhsT=vt_t[r0:r1, c, :],
                                            rhs=p_t[r0:r1,
                                                    pi * IB : (pi + 1) * IB],
                                            start=((c, r0) == first_seg[g]),
                                            stop=((c, r0) == last_seg[g]),
                                        )
                            return pv

                        pending_pv = make_pv()

                    if pending_partial is not None:
                        # A/B tail reached the end of the block; flush the lag
                        pending_pv()
                        pending_pv = None
                        pending_partial()
                        pending_partial = None

                    def make_epilogue_a(accs=accs, h=h, ib=ib, cell=cell):
                        def epilogue_a():
                            comb = comb_pool.tile([DV, IB], f32, tag="comb",
                                                  name=f"comb_{h}_{ib}")
                            t12 = cell["t12"]
                            i_sl = slice(ib * IB, (ib + 1) * IB)
                            if t12 is None and (0 in accs or 1 in accs):
                                t1 = comb_pool.tile([DV, IB], f32, tag="t1",
                                                    name=f"t1f_{h}_{ib}")
                                parts = []
                                if 0 in accs:
                                    nc.vector.tensor_tensor(
                                        t1[:], accs[0][:], wab_t[:, i_sl],
                                        op=MUL)
                                    parts.append(t1)
                                if 1 in accs:
                                    t2 = comb_pool.tile(
                                        [DV, IB], f32, tag="t2",
                                        name=f"t2f_{h}_{ib}")
                                    nc.vector.tensor_tensor(
                                        t2[:], accs[1][:], wbb_t[:, i_sl],
                                        op=MUL)
                                    if parts:
                                        nc.vector.tensor_tensor(
                                            t1[:], t1[:], t2[:], op=ADD)
                                    else:
                                        parts.append(t2)
                                        t1 = t2
                                nc.vector.tensor_tensor(comb[:], t1[:],
                                                        accs[2][:], op=ADD)
                            elif t12 is not None:
                                nc.vector.tensor_tensor(comb[:], t12[:],
                                                        accs[2][:], op=ADD)
                            else:
                                nc.vector.tensor_copy(comb[:], accs[2][:])
                            cell["comb"] = comb
                        return epilogue_a

                    def make_epilogue_b(h=h, ib=ib, cell=cell):
                        def epilogue_b():
                            comb = cell["comb"]
                            for qq in range(IB // P):
                                tr = tr_pool.tile([P, DV], f32, tag="tr",
                                                  name=f"tr_{h}_{ib}_{qq}")
                                nc.tensor.transpose(
                                    tr[:],
                                    comb[:, qq * P : (qq + 1) * P],
                                    ident[:DV, :DV],
                                )
                                rec = out_pool.tile([P, 1], f32, tag="rec",
                                                    name=f"rec_{h}_{ib}_{qq}")
                                nc.vector.reciprocal(
                                    rec[:], tr[:, SUM_ROW : SUM_ROW + 1])
                                o_t = out_pool.tile([P, DH], f32, tag="o",
                                                    name=f"o_{h}_{ib}_{qq}")
                                nc.vector.tensor_scalar_mul(
                                    o_t[:], tr[:, :DH], rec[:])
                                r0 = ib * IB + qq * P
                                nc.sync.dma_start(o_d[h, r0 : r0 + P, :],
                                                  o_t[:])
                        return epilogue_b

                    # flush leftovers (only reachable when pairs-per-block
                    # is small, e.g. tiny-n debug configs)
                    if pending_epilogue is not None:
                        pending_epilogue()
                    if pending_epilogue_b is not None:
                        pending_epilogue_b()
                    pending_epilogue = make_epilogue_a()
                    pending_epilogue_b = make_epilogue_b()

            if pending_pv is not None:
                pending_pv()
            if pending_epilogue is not None:
                pending_epilogue()
            if pending_epilogue_b is not None:
                pending_epilogue_b()

    nc.compile()
    return nc


# ----------------------------------------------------------------------------
# entry point
# ----------------------------------------------------------------------------

def kernel(hidden_states, q, k, v, bboxes, is_cross, ith, num_heads):
    global LAST_RESULTS
    if is_cross:
        return np.asarray(hidden_states)

    from concourse.bass_utils import run_bass_kernel_spmd

    q = np.ascontiguousarray(np.asarray(q, dtype=np.float32))
    k = np.ascontiguousarray(np.asarray(k, dtype=np.float32))
    v = np.ascontiguousarray(np.asarray(v, dtype=np.float32))
    bboxes = np.asarray(bboxes, dtype=np.float32)
    num_heads = int(num_heads)

    bh, n, dh = q.shape
    assert dh == DH and bh % N_CORES == 0 and n % IB == 0
    heads_per_core = bh // N_CORES
    batch = bh // num_heads
    scale = float(1.0 / np.sqrt(np.float32(dh)))

    perm, padded, starts, n_pad, wA, wB = _group_layout(bboxes, n)
    segments = _chunk_segments(padded, starts, n_pad)
    present_groups = sorted({g for segs in segments for (_, _, g) in segs})

    key = (n, n_pad, heads_per_core, tuple(tuple(s) for s in segments))
    if key not in _PROGRAM_CACHE:
        _PROGRAM_CACHE[key] = _build_program(
            n, n_pad, heads_per_core, segments, present_groups, scale
        )
    nc = _PROGRAM_CACHE[key]

    # host-side input prep
    sel = perm >= 0
    kp = np.zeros((bh, n_pad, dh), np.float32)
    kp[:, sel, :] = k[:, perm[sel], :]
    vt = np.zeros((bh, n_pad, DV), np.float32)
    vt[:, sel, :dh] = v[:, perm[sel], :]
    vt[:, sel, SUM_ROW] = 1.0
    kT = np.ascontiguousarray(kp.transpose(0, 2, 1))  # [bh, dh, n_pad]
    qT = np.ascontiguousarray(q.transpose(0, 2, 1))  # [bh, dh, n]
    wab = np.ascontiguousarray(np.broadcast_to(wA[None, :], (DV, n)))
    wbb = np.ascontiguousarray(np.broadcast_to(wB[None, :], (DV, n)))

    in_maps = []
    for c in range(N_CORES):
        sl = slice(c * heads_per_core, (c + 1) * heads_per_core)
        in_maps.append({
            "qT": qT[sl], "kT": kT[sl], "vt": vt[sl],
            "wab": wab, "wbb": wbb, "ident": np.eye(P, dtype=np.float32),
        })

    trace = bool(int(os.environ.get("BASS_ATTN_TRACE", "0")))
    kwargs = {}
    if trace:
        kwargs = dict(trace=True, trace_cores=list(range(N_CORES)))
    res = run_bass_kernel_spmd(nc, in_maps, core_ids=list(range(N_CORES)), **kwargs)
    LAST_RESULTS = res

    out = np.empty((batch, n, num_heads * dh), np.float32)
    for bh_idx in range(bh):
        c, hh = divmod(bh_idx, heads_per_core)
        b, hd = divmod(bh_idx, num_heads)
        out[b, :, hd * dh : (hd + 1) * dh] = res.results[c]["o"][hh]
    return out


# revision 36
# speedup vs baseline: 1.2936x; 1.1128x over previous
"""Masked multi-head self-attention (sparse_attention) on 8 Trainium2 cores.

Strategy
--------
Shard the fused (batch*heads)=16 leading dim of q/k/v across 8 cores, 2 heads
per core.  Per head the kernel computes S^T = K @ Q^T in [j, i] orientation
(128-row j-chunks on partitions, 512-col i-blocks on the free dim), applies
exp on the scalar engine (no max-subtraction needed: |s*scale| <= ~7 so exp
cannot overflow in fp32, and blocked entries are handled structurally, not
additively), then accumulates O^T = V~^T @ P^T on the tensor engine where
V~ = [V | 1] so the softmax denominators fall out of the same matmuls.

The bbox mask has rank-structure: blocked(i,j) <=> (i in A-only and j in
B-only) or vice versa, where A/B are the two subject boxes.  The host sorts
the j (key/value) axis into [A-only | B-only | rest] with 64-aligned zero
padding, so every 64-row half-chunk belongs to one group.  PV matmuls
accumulate into one PSUM accumulator per group; the final combine applies the
per-i 0/1 weights (wA, wB) and sums the three accumulators — the mask costs
no elementwise work on the n*n tiles at all.  Finally each [81, 512] combined
block is PE-transposed back to [128(i), 81], normalized by the sums column,
and DMA'd out in natural i order (the i/query axis is never permuted).
"""

import math
import os

import numpy as np

N_CORES = 8
P = 128  # partitions / j-chunk rows
IB = 512  # i-block width (psum bank, fp32)
DH = 80  # head dim
SUM_ROW = 96  # 32-aligned partition for the sums row (DVE slice rule)
DV = SUM_ROW + 1  # V padded to 96, plus the ones column

_PROGRAM_CACHE = {}
LAST_RESULTS = None  # BassKernelResults of the most recent run (for test.py)


# ----------------------------------------------------------------------------
# host-side mask analysis (mirrors reference._subject_masks / _self_mask)
# ----------------------------------------------------------------------------

def _subject_masks_np(bboxes: np.ndarray, resolution: int) -> np.ndarray:
    b = bboxes[0].astype(np.float32)  # [s, 4]
    x0 = np.round(b[:, 0] * resolution)
    y0 = np.round(b[:, 1] * resolution)
    x1 = np.round(b[:, 2] * resolution)
    y1 = np.round(b[:, 3] * resolution)
    coords = np.arange(resolution, dtype=np.float32)
    xm = (coords[None, :] >= x0[:, None]) & (coords[None, :] < x1[:, None])
    ym = (coords[None, :] >= y0[:, None]) & (coords[None, :] < y1[:, None])
    return (ym[:, :, None] & xm[:, None, :]).reshape(b.shape[0], -1)  # [s, n]


def _group_layout(bboxes: np.ndarray, n: int):
    """Sort the j axis into [A-only | B-only | rest], 64-aligned groups.

    Returns (perm, seg_sizes, group_starts, n_pad, wA, wB) where perm is the
    source index for each padded slot (-1 for zero padding), wA/wB are the
    per-original-i {0,1} combine weights for the A/B accumulators.
    """
    res = int(math.isqrt(n))
    assert res * res == n
    subj = _subject_masks_np(bboxes, res)
    assert subj.shape[0] == 2, "kernel specialized for 2 subject boxes"
    m0, m1 = subj[0], subj[1]
    e0 = m0 & ~m1  # A-only
    e1 = m1 & ~m0  # B-only
    rest = ~(e0 | e1)

    idx = np.arange(n)
    groups = [idx[e0], idx[e1], idx[rest]]

    def ceil64(x):
        return ((x + 63) // 64) * 64

    padded = [ceil64(len(g)) for g in groups]
    n_pad = sum(padded)
    if n_pad % P:
        padded[2] += 64
        n_pad += 64
    perm = np.full(n_pad, -1, dtype=np.int64)
    starts = []
    pos = 0
    for g, plen in zip(groups, padded):
        starts.append(pos)
        perm[pos : pos + len(g)] = g
        pos += plen

    wA = (~e1).astype(np.float32)  # zero the A accumulator for i in B-only
    wB = (~e0).astype(np.float32)  # zero the B accumulator for i in A-only
    return perm, padded, starts, n_pad, wA, wB


def _chunk_segments(padded, starts, n_pad):
    """Per 128-chunk: list of (row_lo, row_hi, group_id) 64-aligned segments."""
    half_group = np.empty(n_pad // 64, dtype=np.int64)
    for gid, (st, plen) in enumerate(zip(starts, padded)):
        half_group[st // 64 : (st + plen) // 64] = gid
    segments = []
    for c in range(n_pad // P):
        g0 = int(half_group[2 * c])
        g1 = int(half_group[2 * c + 1])
        if g0 == g1:
            segments.append([(0, P, g0)])
        else:
            segments.append([(0, 64, g0), (64, P, g1)])
    return segments


# ----------------------------------------------------------------------------
# device program
# ----------------------------------------------------------------------------

def _build_program(n, n_pad, heads_per_core, segments, present_groups, scale):
    import concourse.mybir as mybir
    import concourse.tile as tile
    from concourse import bacc

    f32 = mybir.dt.float32
    f32r = mybir.dt.float32r
    nch = n_pad // P
    n_ib = n // IB
    Exp = mybir.ActivationFunctionType.Exp
    MUL = mybir.AluOpType.mult
    ADD = mybir.AluOpType.add

    nc = bacc.Bacc("TRN2", target_bir_lowering=False, debug=False,
                   num_devices=N_CORES)
    qT_d = nc.dram_tensor("qT", [heads_per_core, DH, n], f32r, kind="ExternalInput")
    kT_d = nc.dram_tensor("kT", [heads_per_core, DH, n_pad], f32r,
                          kind="ExternalInput")
    vt_d = nc.dram_tensor("vt", [heads_per_core, n_pad, DV], f32r,
                          kind="ExternalInput")
    wab_d = nc.dram_tensor("wab", [DV, n], f32, kind="ExternalInput")
    wbb_d = nc.dram_tensor("wbb", [DV, n], f32, kind="ExternalInput")
    id_d = nc.dram_tensor("ident", [P, P], f32, kind="ExternalInput")
    o_d = nc.dram_tensor("o", [heads_per_core, n, DH], f32,
                         kind="ExternalOutput")

    # Chunk processing order: a pure-neutral singleton first (short
    # cross-i-block dependency chain), then every chunk touching the A/B
    # accumulators (so those accumulators finish early and the combine
    # overlaps the long neutral tail), then the remaining neutral chunks.
    ab_chunks = [c for c, segs in enumerate(segments)
                 if any(g != 2 for (_, _, g) in segs)]
    order = []
    rest = list(range(nch))
    if nch % 2:
        order.append((rest.pop(0),))
    order += [tuple(rest[i : i + 2]) for i in range(0, len(rest), 2)]
    last_ab_pair = None  # partial combine disabled (bisect)

    # first/last (chunk, row) PV matmul per group in traversal order
    first_seg = {}
    last_seg = {}
    for pr in order:
        for c in pr:
            for (r0, _, g) in segments[c]:
                first_seg.setdefault(g, (c, r0))
                last_seg[g] = (c, r0)

    with tile.TileContext(nc) as tc:
        with (
            tc.tile_pool(name="const", bufs=1) as const_pool,
            tc.tile_pool(name="head", bufs=2) as head_pool,
            tc.tile_pool(name="p", bufs=3) as p_pool,
            tc.tile_pool(name="comb", bufs=2) as comb_pool,
            tc.tile_pool(name="out", bufs=4) as out_pool,
            tc.tile_pool(name="s_ps", bufs=2, space="PSUM") as s_pool,
            tc.tile_pool(name="acc_ps", bufs=1, space="PSUM") as acc_pool,
            tc.tile_pool(name="tr_ps", bufs=1, space="PSUM") as tr_pool,
        ):
            wab_t = const_pool.tile([DV, n], f32)
            wbb_t = const_pool.tile([DV, n], f32)
            ident = const_pool.tile([P, P], f32)
            nc.sync.dma_start(ident[:], id_d[:])

            # pre-warm the exp table set while the first DMAs run
            warm = const_pool.tile([P, 1], f32)
            nc.vector.memset(warm[:], 0.0)
            nc.scalar.activation(warm[:], warm[:], Exp)

            def load_head(h, eng=None):
                eng = eng or nc.sync
                kT_t = head_pool.tile([DH, nch, P], f32r, tag="kT",
                                      name=f"kT_{h}")
                qT_t = head_pool.tile([DH, n], f32r, tag="qT", name=f"qT_{h}")
                vt_t = head_pool.tile([P, nch, DV], f32r, tag="vt",
                                      name=f"vt_{h}")
                kT_src = kT_d[h].rearrange("d (c j) -> d c j", j=P)
                vt_src = vt_d[h].rearrange("(c p) d -> p c d", p=P)
                # traversal-ordered slices; first slice covers the first pairs
                lead = sorted(set(order[0] + order[1]))
                hi = max(lead) + 1
                cuts = [0, hi]
                for c in (hi + 4, hi + 10, hi + 18, nch):
                    if c > cuts[-1] and c <= nch:
                        cuts.append(min(c, nch))
                if cuts[-1] != nch:
                    cuts.append(nch)
                eng.dma_start(kT_t[:, 0:hi, :], kT_src[:, 0:hi, :])
                eng.dma_start(qT_t[:, 0:IB], qT_d[h][:, 0:IB])
                eng.dma_start(vt_t[:, 0:hi, :], vt_src[:, 0:hi, :])
                ib_next = 1
                for c0, c1 in zip(cuts[1:], cuts[2:]):
                    eng.dma_start(kT_t[:, c0:c1, :], kT_src[:, c0:c1, :])
                    eng.dma_start(vt_t[:, c0:c1, :], vt_src[:, c0:c1, :])
                    if ib_next < n_ib:
                        nc.sync.dma_start(
                            qT_t[:, ib_next * IB : (ib_next + 1) * IB],
                            qT_d[h][:, ib_next * IB : (ib_next + 1) * IB])
                        ib_next += 1
                for ib2 in range(ib_next, n_ib):
                    eng.dma_start(qT_t[:, ib2 * IB : (ib2 + 1) * IB],
                                      qT_d[h][:, ib2 * IB : (ib2 + 1) * IB])
                return kT_t, qT_t, vt_t

            head_tiles = {0: load_head(0)}
            nc.gpsimd.dma_start(wab_t[:], wab_d[:])
            nc.gpsimd.dma_start(wbb_t[:], wbb_d[:])

            pending_epilogue = None
            pending_epilogue_b = None
            pending_pv = None
            consts_loaded = [False]

            for h in range(heads_per_core):
                kT_t, qT_t, vt_t = head_tiles[h]

                for ib in range(n_ib):
                    if ib == 4 and h + 1 < heads_per_core:
                        head_tiles[h + 1] = load_head(h + 1, eng=nc.gpsimd)
                    accs = {
                        g: acc_pool.tile([DV, IB], f32, tag=f"acc{g}",
                                         name=f"acc{g}_{h}_{ib}")
                        for g in present_groups
                    }
                    cell = {}
                    q_sl = qT_t[:, ib * IB : (ib + 1) * IB]

                    def make_partial(accs=accs, h=h, ib=ib, cell=cell):
                        def partial():
                            # A/B accumulators are final: fold them with the
                            # per-i weights now, overlapping the neutral tail
                            i_sl = slice(ib * IB, (ib + 1) * IB)
                            t12 = None
                            if 0 in accs:
                                t1 = comb_pool.tile([DV, IB], f32, tag="t1",
                                                    name=f"t1_{h}_{ib}")
                                nc.vector.tensor_tensor(
                                    t1[:], accs[0][:], wab_t[:, i_sl], op=MUL)
                                t12 = t1
                            if 1 in accs:
                                t2 = comb_pool.tile([DV, IB], f32, tag="t2",
                                                    name=f"t2_{h}_{ib}")
                                nc.vector.tensor_tensor(
                                    t2[:], accs[1][:], wbb_t[:, i_sl], op=MUL)
                                if t12 is None:
                                    t12 = t2
                                else:
                                    nc.vector.tensor_tensor(t12[:], t12[:],
                                                            t2[:], op=ADD)
                            cell["t12"] = t12
                        return partial

                    pending_partial = (make_partial()
                                       if last_ab_pair is not None else None)
                    if pending_partial is None:
                        cell["t12"] = None

                    for t, pr in enumerate(order):
                        s_t = s_pool.tile([P, IB * len(pr)], f32, tag="s")
                        for pi, c in enumerate(pr):
                            nc.tensor.matmul(
                                s_t[:, pi * IB : (pi + 1) * IB],
                                lhsT=kT_t[:, c, :],
                                rhs=q_sl,
                                start=True,
                                stop=True,
                            )
                        p_t = p_pool.tile([P, IB * len(pr)], f32r, tag="p")
                        nc.scalar.activation(p_t[:], s_t[:], Exp, scale=scale)
                        if pending_pv is not None:
                            pending_pv()
                            pending_pv = None
                            if (pending_partial is not None
                                    and t == last_ab_pair + 1):
                                pending_partial()
                                pending_partial = None
                        if t == 2 and pending_epilogue is not None:
                            pending_epilogue()
                            pending_epilogue = None
                        elif t == 4 and pending_epilogue_b is not None:
                            pending_epilogue_b()
                            pending_epilogue_b = None

                        def make_pv(pr=pr, p_t=p_t, accs=accs, vt_t=vt_t):
                            def pv():
                                for pi, c in enumerate(pr):
                                    for (r0, r1, g) in segments[c]:
                                        nc.tensor.matmul(
                                            accs[g][:],
                                            lhsT=vt_t[r0:r1, c, :],
                                            rhs=p_t[r0:r1,
                                                    pi * IB : (pi + 1) * IB],
                                            start=((c, r0) == first_seg[g]),
                                            stop=((c, r0) == last_seg[g]),
                                        )
                            return pv

                        pending_pv = make_pv()

                    if pending_partial is not None:
                        # A/B tail reached the end of the block; flush the lag
                        pending_pv()
                        pending_pv = None
                        pending_partial()
                        pending_partial = None

                    def make_epilogue_a(accs=accs, h=h, ib=ib, cell=cell):
                        def epilogue_a():
                            comb = comb_pool.tile([DV, IB], f32, tag="comb",
                                                  name=f"comb_{h}_{ib}")
                            t12 = cell["t12"]
                            i_sl = slice(ib * IB, (ib + 1) * IB)
                            if t12 is None and (0 in accs or 1 in accs):
                                t1 = comb_pool.tile([DV, IB], f32, tag="t1",
                                                    name=f"t1f_{h}_{ib}")
                                parts = []
                                if 0 in accs:
                                    nc.vector.tensor_tensor(
                                        t1[:], accs[0][:], wab_t[:, i_sl],
                                        op=MUL)
                                    parts.append(t1)
                                if 1 in accs:
                                    t2 = comb_pool.tile(
                                        [DV, IB], f32, tag="t2",
                                        name=f"t2f_{h}_{ib}")
                                    nc.vector.tensor_tensor(
                                        t2[:], accs[1][:], wbb_t[:, i_sl],
                                        op=MUL)
                                    if parts:
                                        nc.vector.tensor_tensor(
                                            t1[:], t1[:], t2[:], op=ADD)
                                    else:
                                        parts.append(t2)
                                        t1 = t2
                                nc.vector.tensor_tensor(comb[:], t1[:],
                                                        accs[2][:], op=ADD)
                            elif t12 is not None:
                                nc.vector.tensor_tensor(comb[:], t12[:],
                                                        accs[2][:], op=ADD)
                            else:
                                nc.vector.tensor_copy(comb[:], accs[2][:])
                            cell["comb"] = comb
                        return epilogue_a

                    def make_epilogue_b(h=h, ib=ib, cell=cell):
                        def epilogue_b():
                            comb = cell["comb"]
                            for qq in range(IB // P):
                                tr = tr_pool.tile([P, DV], f32, tag="tr",
                                                  name=f"tr_{h}_{ib}_{qq}")
                                nc.tensor.transpose(
                                    tr[:],
                                    comb[:, qq * P : (qq + 1) * P],
                                    ident[:DV, :DV],
                                )
                                rec = out_pool.tile([P, 1], f32, tag="rec",
                                                    name=f"rec_{h}_{ib}_{qq}")
                                nc.vector.reciprocal(
                                    rec[:], tr[:, SUM_ROW : SUM_ROW + 1])
                                o_t = out_pool.tile([P, DH], f32, tag="o",
                                                    name=f"o_{h}_{ib}_{qq}")
                                nc.vector.tensor_scalar_mul(
                                    o_t[:], tr[:, :DH], rec[:])
                                r0 = ib * IB + qq * P
                                nc.sync.dma_start(o_d[h, r0 : r0 + P, :],
                                                  o_t[:])
                        return epilogue_b

                    # flush leftovers (only reachable when pairs-per-block
                    # is small, e.g. tiny-n debug configs)
                    if pending_epilogue is not None:
                        pending_epilogue()
                    if pending_epilogue_b is not None:
                        pending_epilogue_b()
                    pending_epilogue = make_epilogue_a()
                    pending_epilogue_b = make_epilogue_b()

            if pending_pv is not None:
                pending_pv()
            if pending_epilogue is not None:
                pending_epilogue()
            if pending_epilogue_b is not None:
                pending_epilogue_b()

    nc.compile()
    return nc


# ----------------------------------------------------------------------------
# entry point
# ----------------------------------------------------------------------------

def kernel(hidden_states, q, k, v, bboxes, is_cross, ith, num_heads):
    global LAST_RESULTS
    if is_cross:
        return np.asarray(hidden_states)

    from concourse.bass_utils import run_bass_kernel_spmd

    q = np.ascontiguousarray(np.asarray(q, dtype=np.float32))
    k = np.ascontiguousarray(np.asarray(k, dtype=np.float32))
    v = np.ascontiguousarray(np.asarray(v, dtype=np.float32))
    bboxes = np.asarray(bboxes, dtype=np.float32)
    num_heads = int(num_heads)

    bh, n, dh = q.shape
    assert dh == DH and bh % N_CORES == 0 and n % IB == 0
    heads_per_core = bh // N_CORES
    batch = bh // num_heads
    scale = float(1.0 / np.sqrt(np.float32(dh)))

    perm, padded, starts, n_pad, wA, wB = _group_layout(bboxes, n)
    segments = _chunk_segments(padded, starts, n_pad)
    present_groups = sorted({g for segs in segments for (_, _, g) in segs})

    key = (n, n_pad, heads_per_core, tuple(tuple(s) for s in segments))
    if key not in _PROGRAM_CACHE:
        _PROGRAM_CACHE[key] = _build_program(
            n, n_pad, heads_per_core, segments, present_groups, scale
        )
    nc = _PROGRAM_CACHE[key]

    # host-side input prep
    sel = perm >= 0
    kp = np.zeros((bh, n_pad, dh), np.float32)
    kp[:, sel, :] = k[:, perm[sel], :]
    vt = np.zeros((bh, n_pad, DV), np.float32)
    vt[:, sel, :dh] = v[:, perm[sel], :]
    vt[:, sel, SUM_ROW] = 1.0
    kT = np.ascontiguousarray(kp.transpose(0, 2, 1))  # [bh, dh, n_pad]
    qT = np.ascontiguousarray(q.transpose(0, 2, 1))  # [bh, dh, n]
    wab = np.ascontiguousarray(np.broadcast_to(wA[None, :], (DV, n)))
    wbb = np.ascontiguousarray(np.broadcast_to(wB[None, :], (DV, n)))

    in_maps = []
    for c in range(N_CORES):
        sl = slice(c * heads_per_core, (c + 1) * heads_per_core)
        in_maps.append({
            "qT": qT[sl], "kT": kT[sl], "vt": vt[sl],
            "wab": wab, "wbb": wbb, "ident": np.eye(P, dtype=np.float32),
        })

    trace = bool(int(os.environ.get("BASS_ATTN_TRACE", "0")))
    kwargs = {}
    if trace:
        kwargs = dict(trace=True, trace_cores=list(range(N_CORES)))
    res = run_bass_kernel_spmd(nc, in_maps, core_ids=list(range(N_CORES)), **kwargs)
    LAST_RESULTS = res

    out = np.empty((batch, n, num_heads * dh), np.float32)
    for bh_idx in range(bh):
        c, hh = divmod(bh_idx, heads_per_core)
        b, hd = divmod(bh_idx, num_heads)
        out[b, :, hd * dh : (hd + 1) * dh] = res.results[c]["o"][hh]
    return out
